# revision 8
# baseline (speedup 1.0000x reference)
"""Trainium2 Bass kernel for nn_CausalFlowModel (encoder MLP -> LSTM-ish scan -> decoder MLP).

Strategy: data-parallel over batch (B=4096 -> 512 per core on 8 cores), all
weights replicated.  Everything on-chip runs in a TRANSPOSED layout
([features, batch]) so the sequential T-loop needs no transposes: the hidden
state hT [H, Bs] is directly the matmul moving operand (rhs) of the next step.

Precision: this system amplifies rounding noise ~1000x through the recurrence,
so low-precision matmuls (fp32r 1-term ~ 11 bits) fail.  The PE's float32r
mode streams fp32 data at full rate (1 cyc/row, vs 4 for real fp32) but
rounds inputs to 11 mantissa bits.  We recover ~fp32 fidelity with a 3-term
split: W = W1 + W2 (static, 11-bit halves), h = h1 + h2 (runtime split: DVE
copy to fp32r = RNE-11, sub exact), and accumulate W1@h1 + W1@h2 + W2@h1 in
fp32 PSUM (dropped W2@h2 ~ 2^-22).  Measured vs fp64: ~6e-4 end-to-end.
The u-path 3-term is free: its K=33 terms are stacked into one K=100 matmul
(cost is N-bound).  Encoder runs plain fp32 matmuls (one-time cost); decoder
runs 1-term fp32r with a 2-term first layer (post-loop, no amplification).

Layouts (per core, Bs = 512 batch columns):
  - state h, c: SBUF [128, 4*512]; free slice j*512:(j+1)*512 = H-tile j
    (h dims j*128..(j+1)*128 on partitions), batch on free dim.
  - preact gates: PSUM [128, 4*512] (4 banks), one tensor per gate.
  - per-step input U[t]: [100, 512]: rows 0-32 u1, 33-65 u2, 66-98 u1, 99 ones
    (pairs with Wu rows Wi1|Wi1|Wi2|bias -> 3-term + bias in one matmul).
  - per-step delta D[t]: [128, 512] = deltas broadcast across partitions.

Gate compute order per step: i, g, o, f (f last -> shortest critical tail
f -> t1=c*f -> c_new -> tanh -> z -> h -> h1/h2 split).  Elementwise tail is
split between DVE (H-tiles 0,1) and GPSIMD (H-tiles 2,3).
"""

import os
import sys
from contextlib import ExitStack

sys.path.insert(0, "/opt/trn_rl_repo")

import numpy as np

import concourse.bass as bass
import concourse.tile as tile
from concourse import bacc, mybir
from concourse.bass_utils import run_bass_kernel_spmd

# ----------------------------------------------------------------------------
# Hardcoded problem shapes
B, T, H, CD, SD = 4096, 128, 512, 32, 64
IN = 1 + CD            # 33
KU = 3 * IN + 1        # 100: stacked u-matmul contraction dim
G = 4 * H              # 2048
NC = 8                 # cores
BS = B // NC           # 512 batch per core
NJ = 4                 # H-tiles (H/128)

AF = mybir.ActivationFunctionType

_BUILD_CACHE = {}


def _tr11(a):
    """Truncate fp32 mantissa to 11 bits (exactly representable in fp32r)."""
    a = np.ascontiguousarray(a, np.float32)
    return (a.view(np.int32) & ~((1 << 12) - 1)).view(np.float32)


def _split11(a):
    hi = _tr11(a)
    lo = (np.asarray(a, np.float32) - hi).astype(np.float32)
    return hi, lo


# Gate order & m-tile bases in the 2048-row preact: i:0-3 f:4-7 o:8-11 g:12-15
GATE_ORDER = ("i", "g", "o", "f")
GATE_MBASE = {"i": 0, "f": 4, "o": 8, "g": 12}
GATE_FUNC = {"i": AF.Sigmoid, "f": AF.Sigmoid, "o": AF.Sigmoid, "g": AF.Tanh}


def build(t_steps=T):
    f32 = mybir.dt.float32
    fr = mybir.dt.float32r

    nc = bacc.Bacc("TRN2", target_bir_lowering=False, debug=False, num_devices=NC)

    # ---- DRAM parameters (per-core shards / replicated weights) ----
    t_dim = max(t_steps, 1)
    xT = nc.declare_dram_parameter("xT", [SD, BS], f32, isOutput=False)
    U = nc.declare_dram_parameter("U", [t_dim, KU, BS], fr, isOutput=False)
    D = nc.declare_dram_parameter("D", [t_dim, 128, BS], f32, isOutput=False)
    Wu = nc.declare_dram_parameter("Wu", [KU, G], fr, isOutput=False)
    Whk1 = nc.declare_dram_parameter("Whk1", [128, NJ, G], fr, isOutput=False)
    Whk2 = nc.declare_dram_parameter("Whk2", [128, NJ, G], fr, isOutput=False)
    e1w = nc.declare_dram_parameter("e1w", [SD, G], f32, isOutput=False)
    e1b = nc.declare_dram_parameter("e1b", [128, 16], f32, isOutput=False)
    e2w = nc.declare_dram_parameter("e2w", [16, 128, 16, 128], f32, isOutput=False)
    e2b = nc.declare_dram_parameter("e2b", [128, 16], f32, isOutput=False)
    e3w = nc.declare_dram_parameter("e3w", [4, 128, 16, 128], f32, isOutput=False)
    e3b = nc.declare_dram_parameter("e3b", [128, 4], f32, isOutput=False)
    d1w = nc.declare_dram_parameter("d1w", [16, 128, 4, 128], fr, isOutput=False)
    d1b = nc.declare_dram_parameter("d1b", [128, 16], f32, isOutput=False)
    d2w = nc.declare_dram_parameter("d2w", [16, 128, 16, 128], fr, isOutput=False)
    d2b = nc.declare_dram_parameter("d2b", [128, 16], f32, isOutput=False)
    d3w = nc.declare_dram_parameter("d3w", [128, 16, SD], fr, isOutput=False)
    d3b = nc.declare_dram_parameter("d3b", [SD, 1], f32, isOutput=False)
    out = nc.declare_dram_parameter("out", [SD, BS], f32, isOutput=True)

    with tile.TileContext(nc) as tc:
        _emit(tc, nc, t_steps, locals())
    nc.compile()
    return nc


def _emit(tc, nc, t_steps, P):
    f32 = mybir.dt.float32
    fr = mybir.dt.float32r
    mm = nc.tensor.matmul

    # ---------------- persistent pools ----------------
    # SBUF budget is tight: h/c are single-buffered (updated in place; the
    # matmuls only ever read the h1/h2 split tiles), and the big RNN weight
    # tensors live in a loop-scoped pool so the encoder can reuse the space.
    stack = ExitStack()
    state_pool = stack.enter_context(tc.tile_pool(name="state", bufs=1))
    hs_pool = stack.enter_context(tc.tile_pool(name="hsplit", bufs=2))

    hA = state_pool.tile([128, NJ * BS], f32, tag="hA")
    cA = state_pool.tile([128, NJ * BS], f32, tag="cA")
    nc.vector.memset(cA[:], 0.0)

    def jsl(ap, j):
        return ap[:, j * BS:(j + 1) * BS]

    # ---------------- encoder (plain fp32 matmuls) ----------------
    with (
        tc.tile_pool(name="enc_sb", bufs=1) as enc_sb,
        tc.tile_pool(name="enc_w", bufs=3) as enc_w,
        tc.tile_pool(name="enc_ps", bufs=8, space="PSUM") as enc_ps,
        tc.tile_pool(name="enc_b", bufs=1) as enc_bp,
    ):
        xT_sb = enc_sb.tile([SD, BS], f32, tag="xT")
        nc.sync.dma_start(xT_sb[:], P["xT"][:])
        e1w_sb = enc_sb.tile([SD, G], f32, tag="e1w")
        nc.sync.dma_start(e1w_sb[:], P["e1w"][:])
        e1b_sb = enc_bp.tile([128, 16], f32, tag="e1b")
        nc.sync.dma_start(e1b_sb[:], P["e1b"][:])
        e2b_sb = enc_bp.tile([128, 16], f32, tag="e2b")
        nc.sync.dma_start(e2b_sb[:], P["e2b"][:])
        e3b_sb = enc_bp.tile([128, 4], f32, tag="e3b")
        nc.sync.dma_start(e3b_sb[:], P["e3b"][:])

        x1T = enc_sb.tile([128, 16, BS], f32, tag="x1T")
        x2T = enc_sb.tile([128, 16, BS], f32, tag="x2T")

        # L1: [64 -> 2048], K = 64
        for m in range(16):
            ps = enc_ps.tile([128, BS], f32)
            mm(ps[:], e1w_sb[:, m * 128:(m + 1) * 128], xT_sb[:],
               start=True, stop=True)
            nc.scalar.activation(x1T[:, m, :], ps[:], AF.Tanh, bias=e1b_sb[:, m:m + 1])

        # L2: [2048 -> 2048], stream m-slabs [128(kp) x 16(k) x 128(M)]
        for m in range(16):
            slab = enc_w.tile([128, 16, 128], f32, tag="e2s")
            nc.sync.dma_start(slab[:], P["e2w"][m])
            ps = enc_ps.tile([128, BS], f32)
            for k in range(16):
                mm(ps[:], slab[:, k, :], x1T[:, k, :],
                   start=(k == 0), stop=(k == 15))
            nc.scalar.activation(x2T[:, m, :], ps[:], AF.Tanh, bias=e2b_sb[:, m:m + 1])

        # L3: [2048 -> 512] -> h0 into state hA (Identity + bias)
        for m in range(4):
            slab = enc_w.tile([128, 16, 128], f32, tag="e3s")
            nc.sync.dma_start(slab[:], P["e3w"][m])
            ps = enc_ps.tile([128, BS], f32)
            for k in range(16):
                mm(ps[:], slab[:, k, :], x2T[:, k, :],
                   start=(k == 0), stop=(k == 15))
            nc.scalar.activation(jsl(hA, m), ps[:], AF.Identity, bias=e3b_sb[:, m:m + 1])

    # initial h1/h2 split of h0
    h1_prev = hs_pool.tile([128, NJ * BS], fr, tag="h1")
    h2_prev = hs_pool.tile([128, NJ * BS], fr, tag="h2")
    nc.vector.tensor_copy(h1_prev[:], hA[:])
    nc.vector.tensor_sub(h2_prev[:], hA[:], h1_prev[:])

    # ---------------- recurrent loop ----------------
    with (
        tc.tile_pool(name="wpool", bufs=1) as wpool,
        tc.tile_pool(name="u_pool", bufs=3) as u_pool,
        tc.tile_pool(name="d_pool", bufs=3) as d_pool,
        tc.tile_pool(name="gates", bufs=1) as gates,
        tc.tile_pool(name="lps", bufs=2, space="PSUM") as lps,
    ):
        wu_sb = wpool.tile([KU, G], fr, tag="wu")
        nc.sync.dma_start(wu_sb[:], P["Wu"][:])
        whk1_sb = wpool.tile([128, NJ, G], fr, tag="whk1")
        nc.sync.dma_start(whk1_sb[:], P["Whk1"][:])
        whk2_sb = wpool.tile([128, NJ, G], fr, tag="whk2")
        nc.sync.dma_start(whk2_sb[:], P["Whk2"][:])

        for t in range(t_steps):
            h_in = h_out = hA
            c_in = c_out = cA

            u_t = u_pool.tile([KU, BS], fr, tag="u")
            nc.sync.dma_start(u_t[:], P["U"][t])
            d_t = d_pool.tile([128, BS], f32, tag="d")
            nc.sync.dma_start(d_t[:], P["D"][t])

            # kk schedule: u-stack first (no h dep), then per H-tile j the
            # three split terms (unblocks as h1_j/h2_j of step t-1 land).
            kk_list = [("u", None, None)]
            for j in range(NJ):
                kk_list += [(whk1_sb, h1_prev, j), (whk1_sb, h2_prev, j),
                            (whk2_sb, h1_prev, j)]
            nkk = len(kk_list)          # 13

            gsb = {}
            for gname in GATE_ORDER:
                mb = GATE_MBASE[gname]
                ps = lps.tile([128, NJ * BS], f32, tag="ps")
                for kk, (w_sb, h_sb, j) in enumerate(kk_list):
                    for mi in range(4):
                        m = mb + mi
                        if isinstance(w_sb, str):
                            lhsT = wu_sb[:, m * 128:(m + 1) * 128]
                            rhs = u_t[:]
                        else:
                            lhsT = w_sb[:, j, m * 128:(m + 1) * 128]
                            rhs = jsl(h_sb, j)
                        mm(jsl(ps, mi), lhsT, rhs,
                           start=(kk == 0), stop=(kk == nkk - 1))
                g_t = gates.tile([128, NJ * BS], f32, tag=gname)
                gsb[gname] = g_t
                if gname in ("i", "g"):
                    nc.scalar.activation(g_t[:], ps[:], GATE_FUNC[gname])
                else:
                    for j in range(NJ):
                        nc.scalar.activation(jsl(g_t, j), jsl(ps, j), GATE_FUNC[gname])

            i_t, g_t, o_t, f_t = gsb["i"], gsb["g"], gsb["o"], gsb["f"]

            h1_cur = hs_pool.tile([128, NJ * BS], fr, tag="h1")
            h2_cur = hs_pool.tile([128, NJ * BS], fr, tag="h2")

            # t2 = i * g (wide, in-place into i)
            nc.vector.tensor_mul(i_t[:], i_t[:], g_t[:])

            # per-H-tile tail; j 0,1 on DVE, j 2,3 on GPSIMD
            for j in range(NJ):
                eng = nc.vector if j < 2 else nc.gpsimd
                eng.tensor_mul(jsl(o_t, j), jsl(o_t, j), d_t[:])        # do_j
            for j in range(NJ):
                eng = nc.vector if j < 2 else nc.gpsimd
                eng.tensor_mul(jsl(f_t, j), jsl(f_t, j), jsl(c_in, j))  # t1_j
                eng.tensor_add(jsl(c_out, j), jsl(f_t, j), jsl(i_t, j))  # c_new_j
            for j in range(NJ):
                nc.scalar.activation(jsl(g_t, j), jsl(c_out, j), AF.Tanh)  # tanh_c_j
            for j in range(NJ):
                eng = nc.vector if j < 2 else nc.gpsimd
                eng.tensor_mul(jsl(g_t, j), jsl(g_t, j), jsl(o_t, j))    # z_j
                eng.tensor_add(jsl(h_out, j), jsl(h_in, j), jsl(g_t, j))  # h_j
                eng.tensor_copy(jsl(h1_cur, j), jsl(h_out, j))           # h1_j (RNE-11)
                eng.tensor_sub(jsl(h2_cur, j), jsl(h_out, j), jsl(h1_cur, j))  # h2_j

            h1_prev, h2_prev = h1_cur, h2_cur

    # ---------------- decoder (fp32r; L1 consumes the final h1/h2) ----------
    with (
        tc.tile_pool(name="dec_sb", bufs=1) as dec_sb,
        tc.tile_pool(name="dec_w", bufs=3) as dec_w,
        tc.tile_pool(name="dec_ps", bufs=8, space="PSUM") as dec_ps,
        tc.tile_pool(name="dec_b", bufs=1) as dec_bp,
    ):
        d1b_sb = dec_bp.tile([128, 16], f32, tag="d1b")
        nc.sync.dma_start(d1b_sb[:], P["d1b"][:])
        d2b_sb = dec_bp.tile([128, 16], f32, tag="d2b")
        nc.sync.dma_start(d2b_sb[:], P["d2b"][:])
        d3b_sb = dec_bp.tile([SD, 1], f32, tag="d3b")
        nc.sync.dma_start(d3b_sb[:], P["d3b"][:])

        y1T = dec_sb.tile([128, 16, BS], fr, tag="y1T")
        y2T = dec_sb.tile([128, 16, BS], fr, tag="y2T")

        # L1: [512 -> 2048], 2-term on the final h split (8 k-MMs per m)
        for m in range(16):
            slab = dec_w.tile([128, 4, 128], fr, tag="d1s")
            nc.sync.dma_start(slab[:], P["d1w"][m])
            ps = dec_ps.tile([128, BS], f32)
            for k in range(4):
                mm(ps[:], slab[:, k, :], jsl(h1_prev, k), start=(k == 0), stop=False)
            for k in range(4):
                mm(ps[:], slab[:, k, :], jsl(h2_prev, k),
                   start=False, stop=(k == 3))
            nc.scalar.activation(y1T[:, m, :], ps[:], AF.Tanh, bias=d1b_sb[:, m:m + 1])

        # L2: [2048 -> 2048]
        for m in range(16):
            slab = dec_w.tile([128, 16, 128], fr, tag="d2s")
            nc.sync.dma_start(slab[:], P["d2w"][m])
            ps = dec_ps.tile([128, BS], f32)
            for k in range(16):
                mm(ps[:], slab[:, k, :], y1T[:, k, :],
                   start=(k == 0), stop=(k == 15))
            nc.scalar.activation(y2T[:, m, :], ps[:], AF.Tanh, bias=d2b_sb[:, m:m + 1])

        # L3: [2048 -> 64]
        d3w_sb = dec_sb.tile([128, 16, SD], fr, tag="d3w")
        nc.sync.dma_start(d3w_sb[:], P["d3w"][:])
        ps = dec_ps.tile([SD, BS], f32)
        for k in range(16):
            mm(ps[:], d3w_sb[:, k, :], y2T[:, k, :],
               start=(k == 0), stop=(k == 15))
        o_sb = dec_sb.tile([SD, BS], f32, tag="out")
        nc.scalar.activation(o_sb[:], ps[:], AF.Identity, bias=d3b_sb[:])
        nc.sync.dma_start(P["out"][:], o_sb[:])

    stack.close()


# ----------------------------------------------------------------------------
def prepare_inputs(x, rnn_input, deltas, Wi, bi, Wh, bh, enc_Ws, enc_bs,
                   dec_Ws, dec_bs, t_steps=T):
    """Host-side shard + layout prep.  Returns in_maps (list of dicts)."""
    f32 = np.float32

    x = np.asarray(x, f32)
    rnn_input = np.asarray(rnn_input, f32)[:t_steps]
    Wi, bi = np.asarray(Wi, f32), np.asarray(bi, f32)
    Wh, bh = np.asarray(Wh, f32), np.asarray(bh, f32)
    enc_Ws = [np.asarray(w, f32) for w in enc_Ws]
    enc_bs = [np.asarray(b, f32) for b in enc_bs]
    dec_Ws = [np.asarray(w, f32) for w in dec_Ws]
    dec_bs = [np.asarray(b, f32) for b in dec_bs]

    # u-path: K-stacked 3-term + bias row
    Wi1, Wi2 = _split11(Wi.T)               # [33, 2048] each
    Wu_h = np.empty((KU, G), f32)
    Wu_h[0:IN] = Wi1
    Wu_h[IN:2 * IN] = Wi1
    Wu_h[2 * IN:3 * IN] = Wi2
    Wu_h[KU - 1] = bi + bh

    # h-path: static 11-bit split of Wh^T, packed [128(kp), 4(k), 2048(m)]
    Wh1, Wh2 = _split11(Wh.T)               # [512, 2048] each
    Whk1_h = np.ascontiguousarray(Wh1.reshape(NJ, 128, G).transpose(1, 0, 2))
    Whk2_h = np.ascontiguousarray(Wh2.reshape(NJ, 128, G).transpose(1, 0, 2))

    e1w_h = np.ascontiguousarray(enc_Ws[0].T)
    e1b_h = np.ascontiguousarray(enc_bs[0].reshape(16, 128).T)
    e2w_h = np.ascontiguousarray(
        enc_Ws[1].T.reshape(16, 128, 16, 128).transpose(2, 1, 0, 3))
    e2b_h = np.ascontiguousarray(enc_bs[1].reshape(16, 128).T)
    e3w_h = np.ascontiguousarray(
        enc_Ws[2].T.reshape(16, 128, 4, 128).transpose(2, 1, 0, 3))
    e3b_h = np.ascontiguousarray(enc_bs[2].reshape(4, 128).T)
    d1w_h = np.ascontiguousarray(
        dec_Ws[0].T.reshape(4, 128, 16, 128).transpose(2, 1, 0, 3))
    d1b_h = np.ascontiguousarray(dec_bs[0].reshape(16, 128).T)
    d2w_h = np.ascontiguousarray(
        dec_Ws[1].T.reshape(16, 128, 16, 128).transpose(2, 1, 0, 3))
    d2b_h = np.ascontiguousarray(dec_bs[1].reshape(16, 128).T)
    d3w_h = np.ascontiguousarray(dec_Ws[2].T.reshape(16, 128, SD).transpose(1, 0, 2))
    d3b_h = np.ascontiguousarray(dec_bs[2].reshape(SD, 1))

    rep = {
        "Wu": Wu_h, "Whk1": Whk1_h, "Whk2": Whk2_h,
        "e1w": e1w_h, "e1b": e1b_h, "e2w": e2w_h, "e2b": e2b_h,
        "e3w": e3w_h, "e3b": e3b_h,
        "d1w": d1w_h, "d1b": d1b_h, "d2w": d2w_h, "d2b": d2b_h,
        "d3w": d3w_h, "d3b": d3b_h,
    }

    xT_g = np.ascontiguousarray(x.T)                            # [64, B]
    uT_g = rnn_input.transpose(0, 2, 1)                         # [T, 33, B]
    u1_g = _tr11(uT_g)
    u2_g = (uT_g - u1_g).astype(f32)
    d_g = rnn_input[:, :, CD]                                   # [T, B]

    t_dim = max(t_steps, 1)
    in_maps = []
    for c in range(NC):
        sl = slice(c * BS, (c + 1) * BS)
        U_h = np.zeros((t_dim, KU, BS), f32)
        U_h[:t_steps, 0:IN] = u1_g[:, :, sl]
        U_h[:t_steps, IN:2 * IN] = u2_g[:, :, sl]
        U_h[:t_steps, 2 * IN:3 * IN] = u1_g[:, :, sl]
        U_h[:t_steps, KU - 1] = 1.0
        D_h = np.zeros((t_dim, 128, BS), f32)
        D_h[:t_steps] = d_g[:t_steps, None, sl]
        m = {"xT": np.ascontiguousarray(xT_g[:, sl]), "U": U_h, "D": D_h}
        m.update(rep)
        in_maps.append(m)
    return in_maps


def run(inputs, t_steps=T, trace=False):
    key = t_steps
    if key not in _BUILD_CACHE:
        _BUILD_CACHE[key] = build(t_steps)
    nc = _BUILD_CACHE[key]
    in_maps = prepare_inputs(t_steps=t_steps, **inputs)
    res = run_bass_kernel_spmd(nc, in_maps, core_ids=list(range(NC)), trace=trace)
    outs = [res.results[c]["out"] for c in range(NC)]           # [64, 512] each
    full = np.concatenate(outs, axis=1).T.astype(np.float32)    # [B, 64]
    return np.ascontiguousarray(full), res


def kernel(**inputs):
    out, _ = run(inputs)
    return out


# revision 10
# speedup vs baseline: 1.1799x; 1.1799x over previous
"""Trainium2 Bass kernel for nn_CausalFlowModel (encoder MLP -> LSTM-ish scan -> decoder MLP).

Strategy: data-parallel over batch (B=4096 -> 512 per core on 8 cores), all
weights replicated.  Everything on-chip runs in a TRANSPOSED layout
([features, batch]) so the sequential T-loop needs no transposes: the hidden
state hT [H, Bs] is directly the matmul moving operand (rhs) of the next step.

Precision: this system amplifies rounding noise ~1000x through the recurrence,
so low-precision matmuls (fp32r 1-term ~ 11 bits) fail.  The PE's float32r
mode streams fp32 data at full rate (1 cyc/row, vs 4 for real fp32) but
rounds inputs to 11 mantissa bits.  We recover ~fp32 fidelity with a 3-term
split: W = W1 + W2 (static, 11-bit halves), h = h1 + h2 (runtime split: DVE
copy to fp32r = RNE-11, sub exact), and accumulate W1@h1 + W1@h2 + W2@h1 in
fp32 PSUM (dropped W2@h2 ~ 2^-22).  Measured vs fp64: ~6e-4 end-to-end.
The u-path 3-term is free: its K=33 terms are stacked into one K=100 matmul
(cost is N-bound).  Encoder runs plain fp32 matmuls (one-time cost); decoder
runs 1-term fp32r with a 2-term first layer (post-loop, no amplification).

Layouts (per core, Bs = 512 batch columns):
  - state h, c: SBUF [128, 4*512]; free slice j*512:(j+1)*512 = H-tile j
    (h dims j*128..(j+1)*128 on partitions), batch on free dim.
  - preact gates: PSUM [128, 4*512] (4 banks), one tensor per gate.
  - per-step input U[t]: [100, 512]: rows 0-32 u1, 33-65 u2, 66-98 u1, 99 ones
    (pairs with Wu rows Wi1|Wi1|Wi2|bias -> 3-term + bias in one matmul).
  - per-step delta D[t]: [128, 512] = deltas broadcast across partitions.

Gate compute order per step: i, g, o, f (f last -> shortest critical tail
f -> t1=c*f -> c_new -> tanh -> z -> h -> h1/h2 split).  Elementwise tail is
split between DVE (H-tiles 0,1) and GPSIMD (H-tiles 2,3).
"""

import os
import sys
from contextlib import ExitStack

sys.path.insert(0, "/opt/trn_rl_repo")

import numpy as np

import concourse.bass as bass
import concourse.tile as tile
from concourse import bacc, mybir
from concourse.bass_utils import run_bass_kernel_spmd

# ----------------------------------------------------------------------------
# Hardcoded problem shapes
B, T, H, CD, SD = 4096, 128, 512, 32, 64
IN = 1 + CD            # 33
KU = 3 * IN + 1        # 100: stacked u-matmul contraction dim
G = 4 * H              # 2048
NC = 8                 # cores
BS = B // NC           # 512 batch per core
NJ = 4                 # H-tiles (H/128)

AF = mybir.ActivationFunctionType

_BUILD_CACHE = {}


def _tr11(a):
    """Truncate fp32 mantissa to 11 bits (exactly representable in fp32r)."""
    a = np.ascontiguousarray(a, np.float32)
    return (a.view(np.int32) & ~((1 << 12) - 1)).view(np.float32)


def _split11(a):
    hi = _tr11(a)
    lo = (np.asarray(a, np.float32) - hi).astype(np.float32)
    return hi, lo


# Gate order & m-tile bases in the 2048-row preact: i:0-3 f:4-7 o:8-11 g:12-15
GATE_ORDER = ("i", "g", "f", "o")
GATE_MBASE = {"i": 0, "f": 4, "o": 8, "g": 12}
GATE_FUNC = {"i": AF.Sigmoid, "f": AF.Sigmoid, "o": AF.Sigmoid, "g": AF.Tanh}


def build(t_steps=T):
    f32 = mybir.dt.float32
    fr = mybir.dt.float32r

    nc = bacc.Bacc("TRN2", target_bir_lowering=False, debug=False, num_devices=NC)

    # ---- DRAM parameters (per-core shards / replicated weights) ----
    t_dim = max(t_steps, 1)
    xT = nc.declare_dram_parameter("xT", [SD, BS], f32, isOutput=False)
    U = nc.declare_dram_parameter("U", [t_dim, KU, BS], fr, isOutput=False)
    D = nc.declare_dram_parameter("D", [t_dim, 128, BS], f32, isOutput=False)
    Wu = nc.declare_dram_parameter("Wu", [KU, G], fr, isOutput=False)
    Whk1 = nc.declare_dram_parameter("Whk1", [128, NJ, G], fr, isOutput=False)
    Whk2 = nc.declare_dram_parameter("Whk2", [128, NJ, G], fr, isOutput=False)
    e1w = nc.declare_dram_parameter("e1w", [SD, G], f32, isOutput=False)
    e1b = nc.declare_dram_parameter("e1b", [128, 16], f32, isOutput=False)
    e2w = nc.declare_dram_parameter("e2w", [16, 128, 16, 128], f32, isOutput=False)
    e2b = nc.declare_dram_parameter("e2b", [128, 16], f32, isOutput=False)
    e3w = nc.declare_dram_parameter("e3w", [4, 128, 16, 128], f32, isOutput=False)
    e3b = nc.declare_dram_parameter("e3b", [128, 4], f32, isOutput=False)
    d1w = nc.declare_dram_parameter("d1w", [16, 128, 4, 128], fr, isOutput=False)
    d1b = nc.declare_dram_parameter("d1b", [128, 16], f32, isOutput=False)
    d2w = nc.declare_dram_parameter("d2w", [16, 128, 16, 128], fr, isOutput=False)
    d2b = nc.declare_dram_parameter("d2b", [128, 16], f32, isOutput=False)
    d3w = nc.declare_dram_parameter("d3w", [128, 16, SD], fr, isOutput=False)
    d3b = nc.declare_dram_parameter("d3b", [SD, 1], f32, isOutput=False)
    out = nc.declare_dram_parameter("out", [SD, BS], f32, isOutput=True)

    with tile.TileContext(nc) as tc:
        _emit(tc, nc, t_steps, locals())
    nc.compile()
    return nc


def _emit(tc, nc, t_steps, P):
    f32 = mybir.dt.float32
    fr = mybir.dt.float32r
    mm = nc.tensor.matmul

    # ---------------- persistent pools ----------------
    # SBUF budget is tight: h/c are single-buffered (updated in place; the
    # matmuls only ever read the h1/h2 split tiles), and the big RNN weight
    # tensors live in a loop-scoped pool so the encoder can reuse the space.
    stack = ExitStack()
    state_pool = stack.enter_context(tc.tile_pool(name="state", bufs=1))
    hs_pool = stack.enter_context(tc.tile_pool(name="hsplit", bufs=2))

    hA = state_pool.tile([128, NJ * BS], f32, tag="hA")
    cA = state_pool.tile([128, NJ * BS], f32, tag="cA")
    nc.vector.memset(cA[:], 0.0)

    def jsl(ap, j):
        return ap[:, j * BS:(j + 1) * BS]

    # ---------------- encoder (plain fp32 matmuls) ----------------
    with (
        tc.tile_pool(name="enc_sb", bufs=1) as enc_sb,
        tc.tile_pool(name="enc_w", bufs=3) as enc_w,
        tc.tile_pool(name="enc_ps", bufs=8, space="PSUM") as enc_ps,
        tc.tile_pool(name="enc_b", bufs=1) as enc_bp,
    ):
        xT_sb = enc_sb.tile([SD, BS], f32, tag="xT")
        nc.sync.dma_start(xT_sb[:], P["xT"][:])
        e1w_sb = enc_sb.tile([SD, G], f32, tag="e1w")
        nc.sync.dma_start(e1w_sb[:], P["e1w"][:])
        e1b_sb = enc_bp.tile([128, 16], f32, tag="e1b")
        nc.sync.dma_start(e1b_sb[:], P["e1b"][:])
        e2b_sb = enc_bp.tile([128, 16], f32, tag="e2b")
        nc.sync.dma_start(e2b_sb[:], P["e2b"][:])
        e3b_sb = enc_bp.tile([128, 4], f32, tag="e3b")
        nc.sync.dma_start(e3b_sb[:], P["e3b"][:])

        x1T = enc_sb.tile([128, 16, BS], f32, tag="x1T")
        x2T = enc_sb.tile([128, 16, BS], f32, tag="x2T")

        # L1: [64 -> 2048], K = 64
        for m in range(16):
            ps = enc_ps.tile([128, BS], f32)
            mm(ps[:], e1w_sb[:, m * 128:(m + 1) * 128], xT_sb[:],
               start=True, stop=True)
            nc.scalar.activation(x1T[:, m, :], ps[:], AF.Tanh, bias=e1b_sb[:, m:m + 1])

        # L2: [2048 -> 2048], stream m-slabs [128(kp) x 16(k) x 128(M)]
        for m in range(16):
            slab = enc_w.tile([128, 16, 128], f32, tag="e2s")
            nc.sync.dma_start(slab[:], P["e2w"][m])
            ps = enc_ps.tile([128, BS], f32)
            for k in range(16):
                mm(ps[:], slab[:, k, :], x1T[:, k, :],
                   start=(k == 0), stop=(k == 15))
            nc.scalar.activation(x2T[:, m, :], ps[:], AF.Tanh, bias=e2b_sb[:, m:m + 1])

        # L3: [2048 -> 512] -> h0 into state hA (Identity + bias)
        for m in range(4):
            slab = enc_w.tile([128, 16, 128], f32, tag="e3s")
            nc.sync.dma_start(slab[:], P["e3w"][m])
            ps = enc_ps.tile([128, BS], f32)
            for k in range(16):
                mm(ps[:], slab[:, k, :], x2T[:, k, :],
                   start=(k == 0), stop=(k == 15))
            nc.scalar.activation(jsl(hA, m), ps[:], AF.Identity, bias=e3b_sb[:, m:m + 1])

    # initial h1/h2 split of h0
    h1_prev = hs_pool.tile([128, NJ * BS], fr, tag="h1")
    h2_prev = hs_pool.tile([128, NJ * BS], fr, tag="h2")
    nc.vector.tensor_copy(h1_prev[:], hA[:])
    nc.vector.tensor_sub(h2_prev[:], hA[:], h1_prev[:])

    # ---------------- recurrent loop ----------------
    with (
        tc.tile_pool(name="wpool", bufs=1) as wpool,
        tc.tile_pool(name="u_pool", bufs=3) as u_pool,
        tc.tile_pool(name="d_pool", bufs=3) as d_pool,
        tc.tile_pool(name="gates", bufs=1) as gates,
        tc.tile_pool(name="lps", bufs=2, space="PSUM") as lps,
    ):
        wu_sb = wpool.tile([KU, G], fr, tag="wu")
        nc.sync.dma_start(wu_sb[:], P["Wu"][:])
        whk1_sb = wpool.tile([128, NJ, G], fr, tag="whk1")
        nc.sync.dma_start(whk1_sb[:], P["Whk1"][:])
        whk2_sb = wpool.tile([128, NJ, G], fr, tag="whk2")
        nc.sync.dma_start(whk2_sb[:], P["Whk2"][:])

        for t in range(t_steps):
            h_in = h_out = hA
            c_in = c_out = cA

            u_t = u_pool.tile([KU, BS], fr, tag="u")
            nc.sync.dma_start(u_t[:], P["U"][t])
            d_t = d_pool.tile([128, BS], f32, tag="d")
            nc.sync.dma_start(d_t[:], P["D"][t])

            # kk schedule: u-stack first (no h dep), then the h1-dependent
            # terms in j order (h1_j lands early), then the h2 terms.
            kk_list = [("u", None, None)]
            for j in range(NJ):
                kk_list += [(whk1_sb, h1_prev, j), (whk2_sb, h1_prev, j)]
            for j in range(NJ):
                kk_list += [(whk1_sb, h2_prev, j)]
            nkk = len(kk_list)          # 13

            gsb = {}
            for gname in GATE_ORDER:
                mb = GATE_MBASE[gname]
                ps = lps.tile([128, NJ * BS], f32, tag="ps")
                for kk, (w_sb, h_sb, j) in enumerate(kk_list):
                    for mi in range(4):
                        m = mb + mi
                        if isinstance(w_sb, str):
                            lhsT = wu_sb[:, m * 128:(m + 1) * 128]
                            rhs = u_t[:]
                        else:
                            lhsT = w_sb[:, j, m * 128:(m + 1) * 128]
                            rhs = jsl(h_sb, j)
                        mm(jsl(ps, mi), lhsT, rhs,
                           start=(kk == 0), stop=(kk == nkk - 1))
                g_t = gates.tile([128, NJ * BS], f32, tag=gname)
                gsb[gname] = g_t
                if gname in ("i", "g"):
                    nc.scalar.activation(g_t[:], ps[:], GATE_FUNC[gname])
                else:
                    for j in range(NJ):
                        nc.scalar.activation(jsl(g_t, j), jsl(ps, j), GATE_FUNC[gname])

            i_t, g_t, f_t, o_t = gsb["i"], gsb["g"], gsb["f"], gsb["o"]

            h1_cur = hs_pool.tile([128, NJ * BS], fr, tag="h1")
            h2_cur = hs_pool.tile([128, NJ * BS], fr, tag="h2")

            # mid-step (off the critical tail):
            #   GPSIMD: t2 = i*g (wide, in-place i); w_j = tanh_c_j * d
            #   DVE:    c-path t1_j = f_j*c_j, c_j = t1_j + t2_j
            #   ACT:    tanh_c_j (into g, free after t2)
            nc.gpsimd.tensor_mul(i_t[:], i_t[:], g_t[:])                 # t2
            for j in range(NJ):
                nc.vector.tensor_mul(jsl(f_t, j), jsl(f_t, j), jsl(c_in, j))   # t1_j
                nc.vector.tensor_add(jsl(c_out, j), jsl(f_t, j), jsl(i_t, j))  # c_new_j
                nc.scalar.activation(jsl(g_t, j), jsl(c_out, j), AF.Tanh)      # tanh_c_j
                nc.gpsimd.tensor_mul(jsl(g_t, j), jsl(g_t, j), d_t[:])         # w_j

            # tail (DVE, ordered for earliest h1_j):
            #   z_j = w_j * o_j ; h1_j = rne11(z_j + h_old_j) via fused STT;
            #   h_j += z_j ; h2_j = h_j - h1_j
            mult = mybir.AluOpType.mult
            add = mybir.AluOpType.add
            tail = [("z", 0), ("h1", 0), ("z", 1), ("h1", 1), ("hf", 0),
                    ("z", 2), ("h1", 2), ("hf", 1), ("z", 3), ("h1", 3),
                    ("hf", 2), ("hf", 3)]
            for op, j in tail:
                if op == "z":
                    nc.vector.tensor_mul(jsl(g_t, j), jsl(g_t, j), jsl(o_t, j))
                elif op == "h1":
                    nc.vector.scalar_tensor_tensor(
                        jsl(h1_cur, j), jsl(g_t, j), 1.0, jsl(h_in, j),
                        op0=mult, op1=add)
                else:  # hf: finalize h and h2
                    nc.vector.tensor_add(jsl(h_out, j), jsl(h_in, j), jsl(g_t, j))
                    nc.vector.tensor_sub(jsl(h2_cur, j), jsl(h_out, j), jsl(h1_cur, j))

            h1_prev, h2_prev = h1_cur, h2_cur

    # ---------------- decoder (fp32r; L1 consumes the final h1/h2) ----------
    with (
        tc.tile_pool(name="dec_sb", bufs=1) as dec_sb,
        tc.tile_pool(name="dec_w", bufs=3) as dec_w,
        tc.tile_pool(name="dec_ps", bufs=8, space="PSUM") as dec_ps,
        tc.tile_pool(name="dec_b", bufs=1) as dec_bp,
    ):
        d1b_sb = dec_bp.tile([128, 16], f32, tag="d1b")
        nc.sync.dma_start(d1b_sb[:], P["d1b"][:])
        d2b_sb = dec_bp.tile([128, 16], f32, tag="d2b")
        nc.sync.dma_start(d2b_sb[:], P["d2b"][:])
        d3b_sb = dec_bp.tile([SD, 1], f32, tag="d3b")
        nc.sync.dma_start(d3b_sb[:], P["d3b"][:])

        y1T = dec_sb.tile([128, 16, BS], fr, tag="y1T")
        y2T = dec_sb.tile([128, 16, BS], fr, tag="y2T")

        # L1: [512 -> 2048], 2-term on the final h split (8 k-MMs per m)
        for m in range(16):
            slab = dec_w.tile([128, 4, 128], fr, tag="d1s")
            nc.sync.dma_start(slab[:], P["d1w"][m])
            ps = dec_ps.tile([128, BS], f32)
            for k in range(4):
                mm(ps[:], slab[:, k, :], jsl(h1_prev, k), start=(k == 0), stop=False)
            for k in range(4):
                mm(ps[:], slab[:, k, :], jsl(h2_prev, k),
                   start=False, stop=(k == 3))
            nc.scalar.activation(y1T[:, m, :], ps[:], AF.Tanh, bias=d1b_sb[:, m:m + 1])

        # L2: [2048 -> 2048]
        for m in range(16):
            slab = dec_w.tile([128, 16, 128], fr, tag="d2s")
            nc.sync.dma_start(slab[:], P["d2w"][m])
            ps = dec_ps.tile([128, BS], f32)
            for k in range(16):
                mm(ps[:], slab[:, k, :], y1T[:, k, :],
                   start=(k == 0), stop=(k == 15))
            nc.scalar.activation(y2T[:, m, :], ps[:], AF.Tanh, bias=d2b_sb[:, m:m + 1])

        # L3: [2048 -> 64]
        d3w_sb = dec_sb.tile([128, 16, SD], fr, tag="d3w")
        nc.sync.dma_start(d3w_sb[:], P["d3w"][:])
        ps = dec_ps.tile([SD, BS], f32)
        for k in range(16):
            mm(ps[:], d3w_sb[:, k, :], y2T[:, k, :],
               start=(k == 0), stop=(k == 15))
        o_sb = dec_sb.tile([SD, BS], f32, tag="out")
        nc.scalar.activation(o_sb[:], ps[:], AF.Identity, bias=d3b_sb[:])
        nc.sync.dma_start(P["out"][:], o_sb[:])

    stack.close()


# ----------------------------------------------------------------------------
def prepare_inputs(x, rnn_input, deltas, Wi, bi, Wh, bh, enc_Ws, enc_bs,
                   dec_Ws, dec_bs, t_steps=T):
    """Host-side shard + layout prep.  Returns in_maps (list of dicts)."""
    f32 = np.float32

    x = np.asarray(x, f32)
    rnn_input = np.asarray(rnn_input, f32)[:t_steps]
    Wi, bi = np.asarray(Wi, f32), np.asarray(bi, f32)
    Wh, bh = np.asarray(Wh, f32), np.asarray(bh, f32)
    enc_Ws = [np.asarray(w, f32) for w in enc_Ws]
    enc_bs = [np.asarray(b, f32) for b in enc_bs]
    dec_Ws = [np.asarray(w, f32) for w in dec_Ws]
    dec_bs = [np.asarray(b, f32) for b in dec_bs]

    # u-path: K-stacked 3-term + bias row
    Wi1, Wi2 = _split11(Wi.T)               # [33, 2048] each
    Wu_h = np.empty((KU, G), f32)
    Wu_h[0:IN] = Wi1
    Wu_h[IN:2 * IN] = Wi1
    Wu_h[2 * IN:3 * IN] = Wi2
    Wu_h[KU - 1] = bi + bh

    # h-path: static 11-bit split of Wh^T, packed [128(kp), 4(k), 2048(m)]
    Wh1, Wh2 = _split11(Wh.T)               # [512, 2048] each
    Whk1_h = np.ascontiguousarray(Wh1.reshape(NJ, 128, G).transpose(1, 0, 2))
    Whk2_h = np.ascontiguousarray(Wh2.reshape(NJ, 128, G).transpose(1, 0, 2))

    e1w_h = np.ascontiguousarray(enc_Ws[0].T)
    e1b_h = np.ascontiguousarray(enc_bs[0].reshape(16, 128).T)
    e2w_h = np.ascontiguousarray(
        enc_Ws[1].T.reshape(16, 128, 16, 128).transpose(2, 1, 0, 3))
    e2b_h = np.ascontiguousarray(enc_bs[1].reshape(16, 128).T)
    e3w_h = np.ascontiguousarray(
        enc_Ws[2].T.reshape(16, 128, 4, 128).transpose(2, 1, 0, 3))
    e3b_h = np.ascontiguousarray(enc_bs[2].reshape(4, 128).T)
    d1w_h = np.ascontiguousarray(
        dec_Ws[0].T.reshape(4, 128, 16, 128).transpose(2, 1, 0, 3))
    d1b_h = np.ascontiguousarray(dec_bs[0].reshape(16, 128).T)
    d2w_h = np.ascontiguousarray(
        dec_Ws[1].T.reshape(16, 128, 16, 128).transpose(2, 1, 0, 3))
    d2b_h = np.ascontiguousarray(dec_bs[1].reshape(16, 128).T)
    d3w_h = np.ascontiguousarray(dec_Ws[2].T.reshape(16, 128, SD).transpose(1, 0, 2))
    d3b_h = np.ascontiguousarray(dec_bs[2].reshape(SD, 1))

    rep = {
        "Wu": Wu_h, "Whk1": Whk1_h, "Whk2": Whk2_h,
        "e1w": e1w_h, "e1b": e1b_h, "e2w": e2w_h, "e2b": e2b_h,
        "e3w": e3w_h, "e3b": e3b_h,
        "d1w": d1w_h, "d1b": d1b_h, "d2w": d2w_h, "d2b": d2b_h,
        "d3w": d3w_h, "d3b": d3b_h,
    }

    xT_g = np.ascontiguousarray(x.T)                            # [64, B]
    uT_g = rnn_input.transpose(0, 2, 1)                         # [T, 33, B]
    u1_g = _tr11(uT_g)
    u2_g = (uT_g - u1_g).astype(f32)
    d_g = rnn_input[:, :, CD]                                   # [T, B]

    t_dim = max(t_steps, 1)
    in_maps = []
    for c in range(NC):
        sl = slice(c * BS, (c + 1) * BS)
        U_h = np.zeros((t_dim, KU, BS), f32)
        U_h[:t_steps, 0:IN] = u1_g[:, :, sl]
        U_h[:t_steps, IN:2 * IN] = u2_g[:, :, sl]
        U_h[:t_steps, 2 * IN:3 * IN] = u1_g[:, :, sl]
        U_h[:t_steps, KU - 1] = 1.0
        D_h = np.zeros((t_dim, 128, BS), f32)
        D_h[:t_steps] = d_g[:t_steps, None, sl]
        m = {"xT": np.ascontiguousarray(xT_g[:, sl]), "U": U_h, "D": D_h}
        m.update(rep)
        in_maps.append(m)
    return in_maps


def run(inputs, t_steps=T, trace=False):
    key = t_steps
    if key not in _BUILD_CACHE:
        _BUILD_CACHE[key] = build(t_steps)
    nc = _BUILD_CACHE[key]
    in_maps = prepare_inputs(t_steps=t_steps, **inputs)
    res = run_bass_kernel_spmd(nc, in_maps, core_ids=list(range(NC)), trace=trace)
    outs = [res.results[c]["out"] for c in range(NC)]           # [64, 512] each
    full = np.concatenate(outs, axis=1).T.astype(np.float32)    # [B, 64]
    return np.ascontiguousarray(full), res


def kernel(**inputs):
    out, _ = run(inputs)
    return out


# revision 13
# speedup vs baseline: 1.3208x; 1.1194x over previous
"""Trainium2 Bass kernel for nn_CausalFlowModel (encoder MLP -> LSTM-ish scan -> decoder MLP).

Strategy: data-parallel over batch (B=4096 -> 512 per core on 8 cores), all
weights replicated.  Everything on-chip runs in a TRANSPOSED layout
([features, batch]) so the sequential T-loop needs no transposes: the hidden
state hT [H, Bs] is directly the matmul moving operand (rhs) of the next step.

Precision: this system amplifies rounding noise ~1000x through the recurrence,
so low-precision matmuls (fp32r 1-term ~ 11 bits) fail.  The PE's float32r
mode streams fp32 data at full rate (1 cyc/row, vs 4 for real fp32) but
rounds inputs to 11 mantissa bits.  We recover ~fp32 fidelity with a 3-term
split: W = W1 + W2 (static, 11-bit halves), h = h1 + h2 (runtime split: DVE
copy to fp32r = RNE-11, sub exact), and accumulate W1@h1 + W1@h2 + W2@h1 in
fp32 PSUM (dropped W2@h2 ~ 2^-22).  Measured vs fp64: ~6e-4 end-to-end.
The u-path 3-term is free: its K=33 terms are stacked into one K=100 matmul
(cost is N-bound).  Encoder runs plain fp32 matmuls (one-time cost); decoder
runs 1-term fp32r with a 2-term first layer (post-loop, no amplification).

Layouts (per core, Bs = 512 batch columns):
  - state h, c: SBUF [128, 4*512]; free slice j*512:(j+1)*512 = H-tile j
    (h dims j*128..(j+1)*128 on partitions), batch on free dim.
  - preact gates: PSUM [128, 4*512] (4 banks), one tensor per gate.
  - per-step input U[t]: [100, 512]: rows 0-32 u1, 33-65 u2, 66-98 u1, 99 ones
    (pairs with Wu rows Wi1|Wi1|Wi2|bias -> 3-term + bias in one matmul).
  - per-step delta D[t]: [128, 512] = deltas broadcast across partitions.

Gate compute order per step: i, g, o, f (f last -> shortest critical tail
f -> t1=c*f -> c_new -> tanh -> z -> h -> h1/h2 split).  Elementwise tail is
split between DVE (H-tiles 0,1) and GPSIMD (H-tiles 2,3).
"""

import os
import sys
from contextlib import ExitStack

sys.path.insert(0, "/opt/trn_rl_repo")

import numpy as np
import ml_dtypes

import concourse.bass as bass
import concourse.tile as tile
from concourse import bacc, mybir
from concourse.bass_utils import run_bass_kernel_spmd

# ----------------------------------------------------------------------------
# Hardcoded problem shapes
B, T, H, CD, SD = 4096, 128, 512, 32, 64
IN = 1 + CD            # 33
KU = 3 * IN + 2        # 101: stacked u-matmul contraction dim (3 u terms + 2 bias rows)
G = 4 * H              # 2048
NC = 8                 # cores
BS = B // NC           # 512 batch per core
NJ = 4                 # H-tiles (H/128)

AF = mybir.ActivationFunctionType

_BUILD_CACHE = {}


def _tr11(a):
    """Truncate fp32 mantissa to 11 bits (exactly representable in fp32r)."""
    a = np.ascontiguousarray(a, np.float32)
    return (a.view(np.int32) & ~((1 << 12) - 1)).view(np.float32)


def _split11(a):
    hi = _tr11(a)
    lo = (np.asarray(a, np.float32) - hi).astype(np.float32)
    return hi, lo


def _bsplit(a):
    """bf16 pair split: a ~= hi + lo with both halves bf16."""
    a = np.asarray(a, np.float32)
    hi = a.astype(ml_dtypes.bfloat16)
    lo = (a - hi.astype(np.float32)).astype(ml_dtypes.bfloat16)
    return hi, lo


# Gate order & m-tile bases in the 2048-row preact: i:0-3 f:4-7 o:8-11 g:12-15
GATE_ORDER = ("i", "g", "f", "o")
GATE_MBASE = {"i": 0, "f": 4, "o": 8, "g": 12}
GATE_FUNC = {"i": AF.Sigmoid, "f": AF.Sigmoid, "o": AF.Sigmoid, "g": AF.Tanh}


def build(t_steps=T):
    f32 = mybir.dt.float32
    fr = mybir.dt.float32r
    bf = mybir.dt.bfloat16

    nc = bacc.Bacc("TRN2", target_bir_lowering=False, debug=False, num_devices=NC)

    # ---- DRAM parameters (per-core shards / replicated weights) ----
    t_dim = max(t_steps, 1)
    xT = nc.declare_dram_parameter("xT", [SD, BS], f32, isOutput=False)
    U = nc.declare_dram_parameter("U", [t_dim, KU, BS], bf, isOutput=False)
    D = nc.declare_dram_parameter("D", [t_dim, 128, BS], f32, isOutput=False)
    Wu = nc.declare_dram_parameter("Wu", [KU, G], bf, isOutput=False)
    Whk1 = nc.declare_dram_parameter("Whk1", [128, NJ, G], bf, isOutput=False)
    Whk2 = nc.declare_dram_parameter("Whk2", [128, NJ, G], bf, isOutput=False)
    e1w = nc.declare_dram_parameter("e1w", [SD, G], f32, isOutput=False)
    e1b = nc.declare_dram_parameter("e1b", [128, 16], f32, isOutput=False)
    e2w = nc.declare_dram_parameter("e2w", [16, 128, 16, 128], f32, isOutput=False)
    e2b = nc.declare_dram_parameter("e2b", [128, 16], f32, isOutput=False)
    e3w = nc.declare_dram_parameter("e3w", [4, 128, 16, 128], f32, isOutput=False)
    e3b = nc.declare_dram_parameter("e3b", [128, 4], f32, isOutput=False)
    d1w = nc.declare_dram_parameter("d1w", [128, 16, 4, 128], fr, isOutput=False)
    d1b = nc.declare_dram_parameter("d1b", [128, 16], f32, isOutput=False)
    d2w = nc.declare_dram_parameter("d2w", [16, 128, 16, 128], fr, isOutput=False)
    d2b = nc.declare_dram_parameter("d2b", [128, 16], f32, isOutput=False)
    d3w = nc.declare_dram_parameter("d3w", [128, 16, SD], fr, isOutput=False)
    d3b = nc.declare_dram_parameter("d3b", [SD, 1], f32, isOutput=False)
    out = nc.declare_dram_parameter("out", [SD, BS], f32, isOutput=True)

    with tile.TileContext(nc) as tc:
        _emit(tc, nc, t_steps, locals())
    nc.compile()
    return nc


def _emit(tc, nc, t_steps, P):
    f32 = mybir.dt.float32
    fr = mybir.dt.float32r
    bf = mybir.dt.bfloat16
    mm = nc.tensor.matmul

    # ---------------- persistent pools ----------------
    # SBUF budget is tight: h/c are single-buffered (updated in place; the
    # matmuls only ever read the h1/h2 split tiles), and the big RNN weight
    # tensors live in a loop-scoped pool so the encoder can reuse the space.
    stack = ExitStack()
    state_pool = stack.enter_context(tc.tile_pool(name="state", bufs=1))
    hs_pool = stack.enter_context(tc.tile_pool(name="hsplit", bufs=2))
    wpool = stack.enter_context(tc.tile_pool(name="wpool", bufs=1))

    # RNN weights + decoder-L1 weights are DMA'd up front so they stream in
    # during encoder compute (no PE stall at the enc->loop / loop->dec seams).
    wu_sb = wpool.tile([KU, G], bf, tag="wu")
    nc.sync.dma_start(wu_sb[:], P["Wu"][:])
    whk1_sb = wpool.tile([128, NJ, G], bf, tag="whk1")
    nc.sync.dma_start(whk1_sb[:], P["Whk1"][:])
    whk2_sb = wpool.tile([128, NJ, G], bf, tag="whk2")
    nc.sync.dma_start(whk2_sb[:], P["Whk2"][:])

    hA = state_pool.tile([128, NJ * BS], f32, tag="hA")
    cA = state_pool.tile([128, NJ * BS], f32, tag="cA")
    nc.vector.memset(cA[:], 0.0)

    def jsl(ap, j):
        return ap[:, j * BS:(j + 1) * BS]

    # ---------------- encoder (plain fp32 matmuls) ----------------
    with (
        tc.tile_pool(name="enc_sb", bufs=1) as enc_sb,
        tc.tile_pool(name="enc_w", bufs=2) as enc_w,
        tc.tile_pool(name="enc_ps", bufs=8, space="PSUM") as enc_ps,
        tc.tile_pool(name="enc_b", bufs=1) as enc_bp,
    ):
        xT_sb = enc_sb.tile([SD, BS], f32, tag="xT")
        nc.sync.dma_start(xT_sb[:], P["xT"][:])
        e1w_sb = enc_sb.tile([SD, G], f32, tag="e1w")
        nc.sync.dma_start(e1w_sb[:], P["e1w"][:])
        e1b_sb = enc_bp.tile([128, 16], f32, tag="e1b")
        nc.sync.dma_start(e1b_sb[:], P["e1b"][:])
        e2b_sb = enc_bp.tile([128, 16], f32, tag="e2b")
        nc.sync.dma_start(e2b_sb[:], P["e2b"][:])
        e3b_sb = enc_bp.tile([128, 4], f32, tag="e3b")
        nc.sync.dma_start(e3b_sb[:], P["e3b"][:])

        x1T = enc_sb.tile([128, 16, BS], f32, tag="x1T")
        x2T = enc_sb.tile([128, 16, BS], f32, tag="x2T")

        # L1: [64 -> 2048], K = 64
        for m in range(16):
            ps = enc_ps.tile([128, BS], f32)
            mm(ps[:], e1w_sb[:, m * 128:(m + 1) * 128], xT_sb[:],
               start=True, stop=True)
            nc.scalar.activation(x1T[:, m, :], ps[:], AF.Tanh, bias=e1b_sb[:, m:m + 1])

        # L2: [2048 -> 2048], stream m-slabs [128(kp) x 16(k) x 128(M)]
        for m in range(16):
            slab = enc_w.tile([128, 16, 128], f32, tag="e2s")
            nc.sync.dma_start(slab[:], P["e2w"][m])
            ps = enc_ps.tile([128, BS], f32)
            for k in range(16):
                mm(ps[:], slab[:, k, :], x1T[:, k, :],
                   start=(k == 0), stop=(k == 15))
            nc.scalar.activation(x2T[:, m, :], ps[:], AF.Tanh, bias=e2b_sb[:, m:m + 1])

        # L3: [2048 -> 512] -> h0 into state hA (Identity + bias)
        for m in range(4):
            slab = enc_w.tile([128, 16, 128], f32, tag="e3s")
            nc.sync.dma_start(slab[:], P["e3w"][m])
            ps = enc_ps.tile([128, BS], f32)
            for k in range(16):
                mm(ps[:], slab[:, k, :], x2T[:, k, :],
                   start=(k == 0), stop=(k == 15))
            nc.scalar.activation(jsl(hA, m), ps[:], AF.Identity, bias=e3b_sb[:, m:m + 1])

    # initial h1/h2 split of h0
    h1_prev = hs_pool.tile([128, NJ * BS], bf, tag="h1")
    h2_prev = hs_pool.tile([128, NJ * BS], bf, tag="h2")
    nc.vector.tensor_copy(h1_prev[:], hA[:])
    nc.vector.tensor_sub(h2_prev[:], hA[:], h1_prev[:])

    # ---------------- recurrent loop ----------------
    with (
        tc.tile_pool(name="u_pool", bufs=3) as u_pool,
        tc.tile_pool(name="d_pool", bufs=3) as d_pool,
        tc.tile_pool(name="gates", bufs=1) as gates,
        tc.tile_pool(name="lps", bufs=2, space="PSUM") as lps,
    ):
        for t in range(t_steps):
            h_in = h_out = hA
            c_in = c_out = cA

            u_t = u_pool.tile([KU, BS], bf, tag="u")
            nc.sync.dma_start(u_t[:], P["U"][t])
            d_t = d_pool.tile([128, BS], f32, tag="d")
            nc.sync.dma_start(d_t[:], P["D"][t])

            # kk schedule: u-stack first (no h dep), then the h1-dependent
            # terms in j order (h1_j lands early), then the h2 terms.
            kk_list = [("u", None, None)]
            for j in range(NJ):
                kk_list += [(whk1_sb, h1_prev, j), (whk2_sb, h1_prev, j)]
            for j in range(NJ):
                kk_list += [(whk1_sb, h2_prev, j)]
            nkk = len(kk_list)          # 13

            def emit_kk(ps, gname, kk):
                mb = GATE_MBASE[gname]
                w_sb, h_sb, j = kk_list[kk]
                for mi in range(4):
                    m = mb + mi
                    if isinstance(w_sb, str):
                        lhsT = wu_sb[:, m * 128:(m + 1) * 128]
                        rhs = u_t[:]
                    else:
                        lhsT = w_sb[:, j, m * 128:(m + 1) * 128]
                        rhs = jsl(h_sb, j)
                    mm(jsl(ps, mi), lhsT, rhs,
                       start=(kk == 0), stop=(kk == nkk - 1))

            gsb = {}
            ps_of = {}
            for gname in GATE_ORDER:
                ps_of[gname] = lps.tile([128, NJ * BS], f32, tag="ps", name=f"ps_{gname}")
            # u-batches of the first two gates run first: they are the only
            # h-independent PE work covering the previous step's split tail.
            emit_kk(ps_of[GATE_ORDER[0]], GATE_ORDER[0], 0)
            emit_kk(ps_of[GATE_ORDER[1]], GATE_ORDER[1], 0)
            for gi, gname in enumerate(GATE_ORDER):
                ps = ps_of[gname]
                for kk in range(1 if gi < 2 else 0, nkk):
                    emit_kk(ps, gname, kk)
                g_t = gates.tile([128, NJ * BS], f32, tag=gname)
                gsb[gname] = g_t
                if gname in ("i", "g"):
                    nc.scalar.activation(g_t[:], ps[:], GATE_FUNC[gname])
                else:
                    for j in range(NJ):
                        nc.scalar.activation(jsl(g_t, j), jsl(ps, j), GATE_FUNC[gname])

            i_t, g_t, f_t, o_t = gsb["i"], gsb["g"], gsb["f"], gsb["o"]

            h1_cur = hs_pool.tile([128, NJ * BS], bf, tag="h1")
            h2_cur = hs_pool.tile([128, NJ * BS], bf, tag="h2")

            # mid-step (off the critical tail):
            #   GPSIMD: t2 = i*g (wide, in-place i); w_j = tanh_c_j * d
            #   DVE:    c-path t1_j = f_j*c_j, c_j = t1_j + t2_j
            #   ACT:    tanh_c_j (into g, free after t2)
            nc.gpsimd.tensor_mul(i_t[:], i_t[:], g_t[:])                 # t2
            for j in range(NJ):
                nc.vector.tensor_mul(jsl(f_t, j), jsl(f_t, j), jsl(c_in, j))   # t1_j
                nc.vector.tensor_add(jsl(c_out, j), jsl(f_t, j), jsl(i_t, j))  # c_new_j
                nc.scalar.activation(jsl(g_t, j), jsl(c_out, j), AF.Tanh)      # tanh_c_j
                nc.gpsimd.tensor_mul(jsl(g_t, j), jsl(g_t, j), d_t[:])         # w_j

            # tail (DVE, ordered for earliest h1_j):
            #   z_j = w_j * o_j ; h1_j = rne11(z_j + h_old_j) via fused STT;
            #   h_j += z_j ; h2_j = h_j - h1_j
            mult = mybir.AluOpType.mult
            add = mybir.AluOpType.add
            tail = [("z", 0), ("h1", 0), ("z", 1), ("h1", 1), ("hf", 0),
                    ("z", 2), ("h1", 2), ("hf", 1), ("z", 3), ("h1", 3),
                    ("hf", 2), ("hf", 3)]
            for op, j in tail:
                if op == "z":
                    nc.vector.tensor_mul(jsl(g_t, j), jsl(g_t, j), jsl(o_t, j))
                elif op == "h1":
                    nc.vector.scalar_tensor_tensor(
                        jsl(h1_cur, j), jsl(g_t, j), 1.0, jsl(h_in, j),
                        op0=mult, op1=add)
                else:  # hf: finalize h and h2
                    nc.vector.tensor_add(jsl(h_out, j), jsl(h_in, j), jsl(g_t, j))
                    nc.vector.tensor_sub(jsl(h2_cur, j), jsl(h_out, j), jsl(h1_cur, j))

            h1_prev, h2_prev = h1_cur, h2_cur

    # ---------------- decoder (fp32r; L1 consumes the final h1/h2) ----------
    with (
        tc.tile_pool(name="dec_sb", bufs=1) as dec_sb,
        tc.tile_pool(name="dec_w", bufs=2) as dec_w,
        tc.tile_pool(name="dec_ps", bufs=8, space="PSUM") as dec_ps,
        tc.tile_pool(name="dec_b", bufs=1) as dec_bp,
    ):
        d1b_sb = dec_bp.tile([128, 16], f32, tag="d1b")
        nc.sync.dma_start(d1b_sb[:], P["d1b"][:])
        d2b_sb = dec_bp.tile([128, 16], f32, tag="d2b")
        nc.sync.dma_start(d2b_sb[:], P["d2b"][:])
        d3b_sb = dec_bp.tile([SD, 1], f32, tag="d3b")
        nc.sync.dma_start(d3b_sb[:], P["d3b"][:])

        y1T = dec_sb.tile([128, 16, BS], fr, tag="y1T")
        y2T = dec_sb.tile([128, 16, BS], fr, tag="y2T")
        h1d = dec_sb.tile([128, NJ * BS], fr, tag="h1d")
        h2d = dec_sb.tile([128, NJ * BS], fr, tag="h2d")
        nc.vector.tensor_copy(h1d[:], hA[:])
        nc.vector.tensor_sub(h2d[:], hA[:], h1d[:])

        # L1: [512 -> 2048], 2-term on the final h split (8 k-MMs per m)
        for m in range(16):
            slab = dec_w.tile([128, 4, 128], fr, tag="d1s")
            nc.sync.dma_start(slab[:], P["d1w"][:, m])
            ps = dec_ps.tile([128, BS], f32)
            for k in range(4):
                mm(ps[:], slab[:, k, :], jsl(h1d, k), start=(k == 0), stop=False)
            for k in range(4):
                mm(ps[:], slab[:, k, :], jsl(h2d, k),
                   start=False, stop=(k == 3))
            nc.scalar.activation(y1T[:, m, :], ps[:], AF.Tanh, bias=d1b_sb[:, m:m + 1])

        # L2: [2048 -> 2048]
        for m in range(16):
            slab = dec_w.tile([128, 16, 128], fr, tag="d2s")
            nc.sync.dma_start(slab[:], P["d2w"][m])
            ps = dec_ps.tile([128, BS], f32)
            for k in range(16):
                mm(ps[:], slab[:, k, :], y1T[:, k, :],
                   start=(k == 0), stop=(k == 15))
            nc.scalar.activation(y2T[:, m, :], ps[:], AF.Tanh, bias=d2b_sb[:, m:m + 1])

        # L3: [2048 -> 64]
        d3w_sb = dec_sb.tile([128, 16, SD], fr, tag="d3w")
        nc.sync.dma_start(d3w_sb[:], P["d3w"][:])
        ps = dec_ps.tile([SD, BS], f32)
        for k in range(16):
            mm(ps[:], d3w_sb[:, k, :], y2T[:, k, :],
               start=(k == 0), stop=(k == 15))
        o_sb = dec_sb.tile([SD, BS], f32, tag="out")
        nc.scalar.activation(o_sb[:], ps[:], AF.Identity, bias=d3b_sb[:])
        nc.sync.dma_start(P["out"][:], o_sb[:])

    stack.close()


# ----------------------------------------------------------------------------
def prepare_inputs(x, rnn_input, deltas, Wi, bi, Wh, bh, enc_Ws, enc_bs,
                   dec_Ws, dec_bs, t_steps=T):
    """Host-side shard + layout prep.  Returns in_maps (list of dicts)."""
    f32 = np.float32

    x = np.asarray(x, f32)
    rnn_input = np.asarray(rnn_input, f32)[:t_steps]
    Wi, bi = np.asarray(Wi, f32), np.asarray(bi, f32)
    Wh, bh = np.asarray(Wh, f32), np.asarray(bh, f32)
    enc_Ws = [np.asarray(w, f32) for w in enc_Ws]
    enc_bs = [np.asarray(b, f32) for b in enc_bs]
    dec_Ws = [np.asarray(w, f32) for w in dec_Ws]
    dec_bs = [np.asarray(b, f32) for b in dec_bs]

    # u-path: K-stacked 3-term + bias pair rows (all bf16)
    Wi1, Wi2 = _bsplit(Wi.T)                # [33, 2048] each, bf16
    b1, b2 = _bsplit(bi + bh)
    bfnp = ml_dtypes.bfloat16
    Wu_h = np.empty((KU, G), bfnp)
    Wu_h[0:IN] = Wi1
    Wu_h[IN:2 * IN] = Wi1
    Wu_h[2 * IN:3 * IN] = Wi2
    Wu_h[KU - 2] = b1
    Wu_h[KU - 1] = b2

    # h-path: static bf16 pair split of Wh^T, packed [128(kp), 4(k), 2048(m)]
    Wh1, Wh2 = _bsplit(Wh.T)                # [512, 2048] each, bf16
    Whk1_h = np.ascontiguousarray(Wh1.reshape(NJ, 128, G).transpose(1, 0, 2))
    Whk2_h = np.ascontiguousarray(Wh2.reshape(NJ, 128, G).transpose(1, 0, 2))

    e1w_h = np.ascontiguousarray(enc_Ws[0].T)
    e1b_h = np.ascontiguousarray(enc_bs[0].reshape(16, 128).T)
    e2w_h = np.ascontiguousarray(
        enc_Ws[1].T.reshape(16, 128, 16, 128).transpose(2, 1, 0, 3))
    e2b_h = np.ascontiguousarray(enc_bs[1].reshape(16, 128).T)
    e3w_h = np.ascontiguousarray(
        enc_Ws[2].T.reshape(16, 128, 4, 128).transpose(2, 1, 0, 3))
    e3b_h = np.ascontiguousarray(enc_bs[2].reshape(4, 128).T)
    d1w_h = np.ascontiguousarray(
        dec_Ws[0].T.reshape(4, 128, 16, 128).transpose(1, 2, 0, 3))
    d1b_h = np.ascontiguousarray(dec_bs[0].reshape(16, 128).T)
    d2w_h = np.ascontiguousarray(
        dec_Ws[1].T.reshape(16, 128, 16, 128).transpose(2, 1, 0, 3))
    d2b_h = np.ascontiguousarray(dec_bs[1].reshape(16, 128).T)
    d3w_h = np.ascontiguousarray(dec_Ws[2].T.reshape(16, 128, SD).transpose(1, 0, 2))
    d3b_h = np.ascontiguousarray(dec_bs[2].reshape(SD, 1))

    rep = {
        "Wu": Wu_h, "Whk1": Whk1_h, "Whk2": Whk2_h,
        "e1w": e1w_h, "e1b": e1b_h, "e2w": e2w_h, "e2b": e2b_h,
        "e3w": e3w_h, "e3b": e3b_h,
        "d1w": d1w_h, "d1b": d1b_h, "d2w": d2w_h, "d2b": d2b_h,
        "d3w": d3w_h, "d3b": d3b_h,
    }

    xT_g = np.ascontiguousarray(x.T)                            # [64, B]
    uT_g = np.ascontiguousarray(rnn_input.transpose(0, 2, 1))   # [T, 33, B]
    u1_g, u2_g = _bsplit(uT_g)
    d_g = rnn_input[:, :, CD]                                   # [T, B]

    t_dim = max(t_steps, 1)
    in_maps = []
    for c in range(NC):
        sl = slice(c * BS, (c + 1) * BS)
        U_h = np.zeros((t_dim, KU, BS), bfnp)
        U_h[:t_steps, 0:IN] = u1_g[:, :, sl]
        U_h[:t_steps, IN:2 * IN] = u2_g[:, :, sl]
        U_h[:t_steps, 2 * IN:3 * IN] = u1_g[:, :, sl]
        U_h[:t_steps, KU - 2] = 1.0
        U_h[:t_steps, KU - 1] = 1.0
        D_h = np.zeros((t_dim, 128, BS), f32)
        D_h[:t_steps] = d_g[:t_steps, None, sl]
        m = {"xT": np.ascontiguousarray(xT_g[:, sl]), "U": U_h, "D": D_h}
        m.update(rep)
        in_maps.append(m)
    return in_maps


def run(inputs, t_steps=T, trace=False):
    key = t_steps
    if key not in _BUILD_CACHE:
        _BUILD_CACHE[key] = build(t_steps)
    nc = _BUILD_CACHE[key]
    in_maps = prepare_inputs(t_steps=t_steps, **inputs)
    res = run_bass_kernel_spmd(nc, in_maps, core_ids=list(range(NC)), trace=trace)
    outs = [res.results[c]["out"] for c in range(NC)]           # [64, 512] each
    full = np.concatenate(outs, axis=1).T.astype(np.float32)    # [B, 64]
    return np.ascontiguousarray(full), res


def kernel(**inputs):
    out, _ = run(inputs)
    return out
